# revision 30
# baseline (speedup 1.0000x reference)
"""Cumulative mean along T (running mean) for input [8, 4096, 1024] f32.

out[b, t, f] = mean(x[b, :t+1, f])

Pure data parallel over batch: 8 cores, one batch element each.
Per core, blocked prefix-sum along T in 128-row blocks (all matmuls f32r =
full-rate single-pass fp32; the input DRAM tensor is declared f32r, which is
bit-identical):

  - main matmul per block: triangular-ones stationary -> psum[t] = local
    prefix(t). Independent across blocks, unrotated output rows.
  - carry chain (the only serial dependency): carry32_{i+1} = carry32_i +
    psum_i[96:128] - [32, FH] DVE adds per block (legal 32-aligned AP base);
    only partition 31 (= psum row 127 = the block total) is meaningful.
    VectorE runs ONLY the chain so hops are never queued behind other work.
  - carry applied for i>0 by a K=32 selector-broadcast matmul accumulating
    into the main PSUM bank: stationary sel[j, t] = 1 iff j == 31, so the PE
    array itself selects the carry row and broadcasts it to all 128 rows.
  - software pipelining: groups of 2 blocks; group g's broadcasts, scales
    and output DMA are emitted AFTER group g+1's main matmuls, so the PE
    stream of mains is paced by input arrival, not by the carry chain
    (PSUM: 2+2 blocks in flight = all 8 banks).
  - per-row 1/(t+1) scale on the Scalar engine (Identity activation with a
    per-partition reciprocal column), which also issues the output DMAs.

DMA (the memory-bound axis): one 1 MiB HWDGE dma_start per 2-block group in
each direction, full 128-partition APs with 4 KiB contiguous rows - measured
~390-415 GB/s sustained. Inputs on the Sync ring, outputs on the Scalar
ring. (Partition-subset or partition-offset output APs collapse write
bandwidth to 45-70 GB/s - keep output DMAs full-partition.)
"""

import numpy as np

import concourse.bacc as bacc
import concourse.tile as tile
from concourse import mybir
from concourse.bass_utils import run_bass_kernel_spmd

B, T, F = 8, 4096, 1024
P = 128
NBLK = T // P  # 32
FH = 512       # one PSUM bank of f32
NHALF = F // FH
CPG = 2        # blocks per DMA group / pipeline stage

F32 = mybir.dt.float32
F32R = mybir.dt.float32r


def _build():
    nc = bacc.Bacc(None, target_bir_lowering=False)
    x_dram = nc.dram_tensor("x", [T, F], F32R, kind="ExternalInput")
    out_dram = nc.dram_tensor("out", [T, F], F32, kind="ExternalOutput")

    lt_np = np.triu(np.ones((P, P), dtype=np.float32))  # lt[s,t]=1 for s<=t
    sel_np = np.zeros((32, P), dtype=np.float32)        # selects carry row 31
    sel_np[31, :] = 1.0
    recip_np = np.ascontiguousarray(
        (1.0 / (np.arange(1, T + 1, dtype=np.float64))).astype(np.float32)
        .reshape(NBLK, P).T
    )  # [p, i] = 1/(i*128+p+1)
    lt_dram = nc.inline_tensor(lt_np, "lt_const")
    sel_dram = nc.inline_tensor(sel_np, "sel_const")
    recip_dram = nc.inline_tensor(recip_np, "recip_const")

    x_rot = x_dram.rearrange("(n p) f -> p n f", p=P)
    out_rot = out_dram.rearrange("(n p) f -> p n f", p=P)

    with tile.TileContext(nc) as tc:
        with (
            tc.tile_pool(name="const", bufs=1) as cpool,
            tc.tile_pool(name="xin", bufs=6) as xpool,
            tc.tile_pool(name="xout", bufs=3) as opool,
            tc.tile_pool(name="run", bufs=6) as rpool,
            tc.tile_pool(name="psum", bufs=4, space="PSUM") as ppool,
        ):
            lt_f32 = cpool.tile([P, P], F32)
            nc.gpsimd.dma_start(lt_f32[:], lt_dram[:])
            lt = cpool.tile([P, P], F32R)
            nc.vector.tensor_copy(lt[:], lt_f32[:])
            sel_f32 = cpool.tile([32, P], F32)
            nc.gpsimd.dma_start(sel_f32[:], sel_dram[:])
            sel = cpool.tile([32, P], F32R)
            nc.vector.tensor_copy(sel[:], sel_f32[:])
            recip = cpool.tile([P, NBLK], F32)
            nc.gpsimd.dma_start(recip[:], recip_dram[:])

            def flush(pend):
                psums, carries, pbase, pgsz = pend
                ot = opool.tile([P, CPG, F], F32, tag="ot")
                for c in range(pgsz):
                    if carries[c] is not None:
                        for h in range(NHALF):
                            hs = slice(h * FH, (h + 1) * FH)
                            nc.tensor.matmul(
                                psums[c][:, hs], sel[:], carries[c][:, hs],
                                start=False, stop=True,
                            )
                for c in range(pgsz):
                    i = pbase + c
                    nc.scalar.activation(
                        ot[:, c, :], psums[c][:],
                        mybir.ActivationFunctionType.Identity,
                        scale=recip[:, i : i + 1],
                    )
                nc.scalar.dma_start(
                    out_rot[:, pbase : pbase + pgsz, :], ot[:, 0:pgsz, :]
                )

            carry = None  # [32, F] f32r; partition 31 = sum of blocks < i
            pend = None
            base = 0
            for g in range(NBLK // CPG):
                xt = xpool.tile([P, CPG, F], F32R, tag="xt")
                nc.sync.dma_start(xt[:], x_rot[:, base : base + CPG, :])

                psums = []
                carries = []
                for c in range(CPG):
                    i = base + c
                    ps = ppool.tile([P, F], F32)
                    psums.append(ps)
                    carries.append(carry)
                    for h in range(NHALF):
                        hs = slice(h * FH, (h + 1) * FH)
                        nc.tensor.matmul(
                            ps[:, hs], lt[:], xt[:, c, hs],
                            start=True, stop=(i == 0),
                        )
                    # Carry chain hop (VectorE), reading local prefix rows
                    # 96..127 before the deferred broadcast matmul rewrites
                    # the bank.
                    if i < NBLK - 1:
                        new_carry = rpool.tile([32, F], F32R)
                        for h in range(NHALF):
                            hs = slice(h * FH, (h + 1) * FH)
                            if carry is None:
                                nc.vector.tensor_copy(
                                    new_carry[:, hs], ps[96:P, hs]
                                )
                            else:
                                nc.vector.tensor_tensor(
                                    new_carry[:, hs],
                                    carry[:, hs].bitcast(F32),
                                    ps[96:P, hs],
                                    mybir.AluOpType.add,
                                )
                        carry = new_carry

                if pend is not None:
                    flush(pend)
                pend = (psums, carries, base, CPG)
                base += CPG

            flush(pend)

    nc.compile()
    return nc


_NC_CACHE = None
last_results = None  # BassKernelResults of the most recent run (for test harness)


def kernel(inputs: np.ndarray) -> np.ndarray:
    global _NC_CACHE, last_results
    if _NC_CACHE is None:
        _NC_CACHE = _build()
    nc = _NC_CACHE
    x = np.ascontiguousarray(np.asarray(inputs, dtype=np.float32))
    assert x.shape == (B, T, F), x.shape
    in_maps = [{"x": x[b]} for b in range(B)]
    res = run_bass_kernel_spmd(nc, in_maps, core_ids=list(range(B)))
    last_results = res
    return np.stack([r["out"] for r in res.results], axis=0)
